# revision 111
# baseline (speedup 1.0000x reference)
"""Trainium2 Bass kernel: multi-head attention forward (B=2, S=2048, D=1024, H=16).

Sharding: 8 cores = data-parallel over batch (2) x tensor-parallel over heads
(4 head-groups of 4 heads).  Host side: inputs are pre-transposed / sliced /
fp8-split per core; the 4 partial outputs per batch are summed and the bias
added on the host (the "unshard").

Precision scheme (measured rel err 1.21e-2 vs the 2e-2 gate):
  - x and Wq/Wk/Wv are hi/lo-split into e4m3 pairs on the host
    (t = hi + lo with hi = fp8(t), lo = fp8(t - hi); residual ~0.1%).
    Weights are pre-scaled by 32 so the lo parts stay in e4m3's normal
    range; the scale is compensated in the exp scale (/1024) and in
    Wo (/32).
  - q/k projections: 2-term fp8 DoubleRow (xh@(Wh+Wl)): exact weights,
    single-fp8 x; q is re-quantized to e4m3 at the scores and k is
    re-split exactly, so an xl term would add nothing.  8 DR
    instructions of N/2 cycles vs 8 of N for f32r (0.5x PE).
  - v projection: 3-term (xh@Wvh + xh@Wvl + xl@Wvh, 0.75x); v errors do
    not average out downstream, so single-fp8 x is NOT acceptable here
    (measured 3e-2).
  - scores: one DoubleRow per sk-tile: stationary slots (kh, kl) so k is
    exact; moving q is a single e4m3 (slot-broadcast AP) -> 0.5x PE cost.
  - exp -> bf16 attention weights (ACT), processed in ski-pairs to halve
    the per-instruction overhead.
  - PV and output projection in bf16 (same PE cost as f32r); bf16 output
    partials, summed in f32 on the host.
  - softmax denominators via a bf16 ones-column in v (row 64 of the PV
    psum); normalization: DVE copy + fast reciprocal + Pool broadcast +
    DVE multiply (DVE divide fails the walrus ISA check; GPSIMD cannot
    read PSUM on hardware).

Scheduling: a global software-pipelined pass over (block, head, ski-pair)
steps: scores+exp run one pair ahead of PV; projection and out-projection
work is split into sub-0.5us filler units paced by a static ACT/PE clock
model so the exp stream (the ~78us ACT floor) never starves; per-block ctx
tiles avoid coarse-dep serialization; DMAs are ordered by first use and
weights host-packed to 2KB-contiguous partition runs (sub-512B runs pay a
2x DMA penalty).
"""

import sys

sys.path.insert(0, "/opt/trn_rl_repo")

import numpy as np
import ml_dtypes

B, S, D = 2, 2048, 1024
H = 16
DH = 64
HL = 4  # heads per core
NCORES = 8

WSC = 32.0  # power-of-2 weight prescale for fp8 range
E4 = ml_dtypes.float8_e4m3
BF = ml_dtypes.bfloat16

_PROGRAM_CACHE = {}


def build_program(S=S, D=D, HL=HL, DH=DH):
    import concourse.tile as tile
    from concourse import bacc, mybir

    f32 = mybir.dt.float32
    f8 = mybir.dt.float8e4
    bf16 = mybir.dt.bfloat16
    A = mybir.ActivationFunctionType
    Alu = mybir.AluOpType
    DR = mybir.MatmulPerfMode.DoubleRow

    K2T = True            # 2-term k-projection (k re-split exactly at scores)
    DIVN = False          # DVE divide fails the walrus ISA check on TRN2
    KD = D // 128         # contraction k-tiles for the projections (8)
    NP = KD // 2          # DoubleRow k-tile pairs (4)
    M = HL * DH           # per-core projected width (256)
    MQ = M // 128         # q/k partition planes (2)
    ST = S // 128         # 128-row s tiles (16)
    W = min(512, S)       # query-block width
    NJ = S // W           # query blocks
    TPB = W // 128        # sk tiles per query block (4)
    scale = (1.0 / float(np.sqrt(DH))) / (WSC * WSC)

    nc = bacc.Bacc("TRN2", target_bir_lowering=False, debug=False)
    xh = nc.dram_tensor("xh", (D, S), f8, kind="ExternalInput").ap()
    xl = nc.dram_tensor("xl", (D, S), f8, kind="ExternalInput").ap()
    # weights pre-packed on the host to [128, KD*M] so each partition's
    # DMA run is 2KB contiguous (256B runs pay a 2x DMA latency penalty)
    wnames = ("wqh", "wql", "wkh", "wkl", "wvh", "wvl")
    wt = {n: nc.dram_tensor(n, (128, KD * M), f8, kind="ExternalInput").ap()
          for n in wnames}
    wo = nc.dram_tensor("wo", (M, D), bf16, kind="ExternalInput").ap()
    # bf16 output: DMA transfers serialize on one engine, so halving the
    # 8MB store matters; the host sums the 4 partials in f32
    out = nc.dram_tensor("out", (S, D), bf16, kind="ExternalOutput").ap()

    xh_r = xh.rearrange("(k p) s -> p k s", p=128)
    xl_r = xl.rearrange("(k p) s -> p k s", p=128)
    w_r = {n: wt[n].rearrange("p (k m) -> p k m", k=KD) for n in wnames}

    with tile.TileContext(nc) as tc:
        with (
            tc.tile_pool(name="persist", bufs=1) as mp,
            tc.tile_pool(name="attn", bufs=4) as apool,
            tc.tile_pool(name="norm", bufs=2) as rpool,
            tc.tile_pool(name="ostage", bufs=8) as opool,
            tc.tile_pool(name="gps", bufs=2, space="PSUM") as gpool,
            tc.tile_pool(name="scps", bufs=2, space="PSUM") as spool,
            tc.tile_pool(name="ctxps", bufs=2, space="PSUM") as cpool,
        ):
            xh_sb = mp.tile([128, KD, S], f8, tag="xh")
            xl_sb = mp.tile([128, KD, S], f8, tag="xl")
            w_sb = {n: mp.tile([128, KD, M], f8, tag=n, name=n) for n in w_r}
            wo_sb = mp.tile([128, MQ, D], bf16, tag="wo")
            q8_sb = mp.tile([128, MQ, S], f8, tag="q8")
            k8_sb = mp.tile([128, MQ, 2, S], f8, tag="k8")
            v_sb = mp.tile([128, ST, HL * (DH + 1)], bf16, tag="v")
            # per-block ctx tiles: the dep tracker is coarse per tile, so a
            # single [128, MQ, S] tile would serialize block j's out-proj
            # behind block j+1's normalization writes
            ctx_sb = [
                mp.tile([128, MQ, W], bf16, tag=f"ctx{j}", name=f"ctx{j}")
                for j in range(NJ)
            ]

            # ones columns for the PV denominator trick
            nc.vector.memset(v_sb[:, :, DH::DH + 1], 1.0)

            # pstate warmup: dummy matmuls while the first DMAs are in
            # flight, so real work starts at full PE clock
            wu = mp.tile([128, 2, 128], f8, tag="wu")
            nc.vector.memset(wu[:], 0.0)
            wu_ps = gpool.tile([128, 512], f32, tag="ps", name="wu_ps")
            nc.tensor.matmul(
                wu_ps[:, 0:128],
                wu[:, :, :],
                wu[:, :, :],
                start=True, stop=True, perf_mode=DR,
            )

            # ------------- per-block emitters ---------------------------
            def emit_x_dmas(j, part="hl"):
                jsl = slice(j * W, (j + 1) * W)
                if "h" in part:
                    nc.sync.dma_start(xh_sb[:, :, jsl], xh_r[:, :, jsl])
                if "l" in part:
                    nc.sync.dma_start(xl_sb[:, :, jsl], xl_r[:, :, jsl])

            def qk_units(j, m, dst, q8_on_act=False):
                """The q/k projection group as two half units sharing one
                psum accumulation (filler granularity below the per-pair PE
                slack, so emitting one never starves the exp stream)."""
                jsl = slice(j * W, (j + 1) * W)
                msl = slice(m * 128, (m + 1) * 128)
                wh, wl = ("wqh", "wql") if dst == "q" else ("wkh", "wkl")
                state = {}

                def unit(u):
                    if u == 0:
                        state["ps"] = gpool.tile([128, W], f32, tag="ps",
                                                 name="ps_qk")
                    ps = state["ps"]
                    # 2-term projection (exact W, single-fp8 x): q is
                    # re-quantized to e4m3 at the scores and k is re-split
                    # exactly, so the xl term would be wasted effort
                    i = 4 * u
                    for p in (2 * u, 2 * u + 1):
                        psl = slice(2 * p, 2 * p + 2)
                        for sw in (wh, wl):
                            nc.tensor.matmul(
                                ps[:],
                                w_sb[sw][:, psl, msl],
                                xh_sb[:, psl, jsl],
                                start=(i == 0),
                                stop=(i == 2 * NP - 1),
                                perf_mode=DR,
                            )
                            i += 1
                    if u == 1:
                        if dst == "q":
                            if q8_on_act:
                                # ACT is idle before the exp stream starts:
                                # unserializes the first q8/kh/kl copies
                                nc.scalar.copy(q8_sb[:, m, jsl], ps[:])
                            else:
                                nc.vector.tensor_copy(q8_sb[:, m, jsl], ps[:])
                        else:
                            nc.vector.tensor_copy(k8_sb[:, m, 0, jsl], ps[:])
                            nc.vector.tensor_sub(
                                k8_sb[:, m, 1, jsl], ps[:],
                                k8_sb[:, m, 0, jsl],
                            )

                return [lambda u=u: unit(u) for u in range(2)]

            def v_units(st):
                ssl = slice(st * 128, st * 128 + 128)
                terms = [(xh_sb, "wvh"), (xh_sb, "wvl"), (xl_sb, "wvh")]
                state = {}

                def unit(u):
                    if u == 0:
                        state["ps"] = gpool.tile([128, M], f32, tag="ps",
                                                 name="psv")
                    psv = state["ps"]
                    todo = [(t, p) for t in range(3) for p in range(NP)]
                    for t, p in todo[6 * u:6 * (u + 1)]:
                        sx, sw = terms[t]
                        psl = slice(2 * p, 2 * p + 2)
                        nc.tensor.matmul(
                            psv[:],
                            sx[:, psl, ssl],
                            w_sb[sw][:, psl, :],
                            start=(t == 0 and p == 0),
                            stop=(t == 2 and p == NP - 1),
                            perf_mode=DR,
                        )
                    if u == 1:
                        vdst = v_sb[:, st].rearrange(
                            "p (h c) -> p h c", h=HL)[:, :, 0:DH]
                        nc.vector.tensor_copy(
                            vdst, psv[:].rearrange("p (h c) -> p h c", h=HL))

                return [lambda u=u: unit(u) for u in range(2)]

            def emit_qk_group(j, m, dst, q8_on_act=False):
                for fn in qk_units(j, m, dst, q8_on_act):
                    fn()

            def emit_v_group(st):
                for fn in v_units(st):
                    fn()

            def emit_outproj_group(st, n, copy_dve=True, dma_act=False,
                                   nw=512, pool=None):
                ssl = slice(st * 128, st * 128 + 128)
                csl = slice((st % TPB) * 128, (st % TPB) * 128 + 128)
                nsl = slice(n * nw, (n + 1) * nw)
                if pool is None:
                    ops = gpool.tile([128, 512], f32, tag="ps", name="ops")
                else:
                    # tail: borrow the (idle) scores-pool slots so the drain
                    # has a 4-deep psum ring instead of 2
                    ops = pool.tile([128, 2, W], f32, tag="sc",
                                    name="ops_sc")[:, 0, :]
                for p2 in range(MQ):
                    nc.tensor.matmul(
                        ops[:, 0:nw],
                        ctx_sb[st // TPB][:, p2, csl],
                        wo_sb[:, p2, nsl],
                        start=(p2 == 0),
                        stop=(p2 == MQ - 1),
                    )
                # GPSIMD cannot read PSUM on hardware: stage on DVE (or ACT
                # for the drain, where the exps are done)
                o_sb = opool.tile([128, 512], bf16, tag="o")
                if copy_dve:
                    nc.vector.tensor_copy(o_sb[:, 0:nw], ops[:, 0:nw])
                else:
                    nc.scalar.copy(o_sb[:, 0:nw], ops[:, 0:nw])
                if dma_act:
                    nc.scalar.dma_start(out[ssl, nsl], o_sb[:, 0:nw])
                else:
                    nc.sync.dma_start(out[ssl, nsl], o_sb[:, 0:nw])

            # fillers: PE work (next blocks' projections, previous blocks'
            # output projection) with deadlines, spread evenly over the
            # global attention pair-step sequence to fill PE stalls on ACT
            def run_attention_steps(tail_pre=None):
                def emit_scores_pair(j, h, skp):
                    """scores + exp + causal mask for one ski-pair; returns
                    what the (deferred) PV pair needs."""
                    hm, po = h // 2, 64 * (h % 2)
                    qrow = slice(po, po + DH)
                    sc = spool.tile([128, 2, W], f32, tag="sc")
                    ex = [
                        max(0, 128 * (skp + t) - j * W)
                        if skp + t >= TPB * j else 0
                        for t in range(2)
                    ]
                    for t in range(2):
                        ski = skp + t
                        # both slots cover the pair's union range so the
                        # paired exp reads fully-written psum; the extra
                        # columns in slot 1 are causally dead (PV skips)
                        q_mov = q8_sb[qrow, hm:hm + 1,
                                      j * W + ex[0]:(j + 1) * W]
                        nc.tensor.matmul(
                            sc[:, t, ex[0]:W],
                            k8_sb[qrow, hm, :, ski * 128:ski * 128 + 128],
                            q_mov.broadcast_to((DH, 2, W - ex[0])),
                            start=True,
                            stop=True,
                            perf_mode=DR,
                        )
                    attn = apool.tile([128, 2, W], bf16, tag="attn")
                    nc.scalar.activation(
                        attn[:, :, ex[0]:W], sc[:, :, ex[0]:W], A.Exp,
                        scale=scale,
                    )
                    for t in range(2):
                        ski = skp + t
                        if ski >= TPB * j:  # diagonal-crossing tile
                            cross_end = 128 * ski + 128 - j * W
                            nc.gpsimd.affine_select(
                                out=attn[:, t, ex[t]:cross_end],
                                in_=attn[:, t, ex[t]:cross_end],
                                compare_op=Alu.is_ge,
                                fill=0.0,
                                base=j * W + ex[t] - 128 * ski,
                                pattern=[[1, cross_end - ex[t]]],
                                channel_multiplier=-1,
                            )
                    return attn, ex

                def emit_pv_pair(ctx_ps, j, h, skp, attn, ex):
                    hv = slice(h * (DH + 1), (h + 1) * (DH + 1))
                    fs, ls = bounds[(j, h)]
                    for t in range(2):
                        ski = skp + t
                        nc.tensor.matmul(
                            ctx_ps[:, ex[t]:W],
                            v_sb[:, ski, hv],
                            attn[:, t, ex[t]:W],
                            start=(ski == fs),
                            stop=(ski == ls),
                        )

                def emit_norm(ctx_ps, j, h, halves=1):
                    hm, po = h // 2, 64 * (h % 2)
                    hw = W // halves
                    for c in range(halves):
                        cs = slice(c * hw, (c + 1) * hw)
                        dcp = rpool.tile([1, W], f32, tag="d")
                        rcp = rpool.tile([1, W], f32, tag="r")
                        bc = rpool.tile([64, W], f32, tag="bc")
                        nc.vector.tensor_copy(dcp[:, 0:hw], ctx_ps[DH:DH + 1, cs])
                        nc.vector.reciprocal_approx_fast(
                            out=rcp[:, 0:hw], in_=dcp[:, 0:hw])
                        nc.gpsimd.partition_broadcast(
                            bc[:, 0:hw], rcp[:, 0:hw], channels=64)
                        nc.vector.tensor_mul(
                            ctx_sb[j][po:po + DH, hm, cs], ctx_ps[0:DH, cs],
                            bc[:, 0:hw],
                        )

                # global step list over (block, head, ski-pair).  For the
                # final head the diagonal pairs go FIRST (psum accumulation
                # order is free) so no affine_select sits on the critical
                # chain right before the last PV + softmax.
                steps = []
                bounds = {}
                for j in range(NJ):
                    for h in range(HL):
                        prs = list(range(0, TPB * (j + 1), 2))
                        if (j, h) == (NJ - 1, HL - 1) and j > 0:
                            d = TPB * j // 2
                            prs = prs[d:] + prs[:d]
                        bounds[(j, h)] = (prs[0], prs[-1] + 1)
                        steps += [(j, h, skp) for skp in prs]
                sidx = {s: i for i, s in enumerate(steps)}
                n_steps = len(steps)

                # fillers: (earliest, deadline, unit_pe_ns, [unit closures]);
                # block 0's second-plane q/k and v groups are fillers too so
                # the first scores (and the exp stream) start sooner.
                # Pacing tracks a static model of the ACT and PE clocks and
                # emits one sub-slack-sized unit whenever PE would otherwise
                # run ahead of the exp stream.  A group's units share a psum
                # tile, so once started a group is finished before any other
                # filler (nothing else touches that pool between pace calls).
                cyc = 1.0 / 2.4
                qk_ns = 4 * 256 * cyc
                v_ns = 6 * 128 * cyc
                op_ns = 2 * 512 * cyc
                fillers = []
                for j in range(1, NJ):
                    e0 = sidx[(j - 1, 0, 0)]
                    for m in range(MQ):
                        dl = sidx[(j, 2 * m, 0)]
                        fillers.append([e0, dl, qk_ns, qk_units(j, m, "q")])
                        fillers.append([e0, dl, qk_ns, qk_units(j, m, "k")])
                    for st in range(j * TPB, (j + 1) * TPB):
                        dl = sidx[(j, 0, st - st % 2)] + 1
                        fillers.append([e0, dl, v_ns, v_units(st)])
                for j in range(NJ - 1):
                    # not at (j+1, 0, 0): block j's last norm-mul is emitted
                    # there, and an out-proj filler would head-of-line block
                    # the PE queue waiting for it, starving the exp stream
                    e0 = sidx[(j + 1, 1, 0)]
                    for st in range(j * TPB, (j + 1) * TPB):
                        for n in range(D // 512):
                            fillers.append([e0, n_steps, op_ns,
                                            [lambda st=st, n=n:
                                             emit_outproj_group(st, n)]])
                fillers.sort(key=lambda f: f[1])
                # PE starts the step loop still working off the upfront
                # block-0 projections; pre-load its clock so early pacing
                # does not over-fill and starve the exp stream
                clock = [0.0, 4 * 2 * qk_ns + 4 * 2 * v_ns]
                open_g = []  # remaining units of the group being emitted

                def emit_forced(i):
                    # spread deadline work: emit only as many units per step
                    # as needed to make each deadline, instead of dumping
                    # whole groups in one burst that starves the exp stream
                    need = any(f[3] is not None and len(f[3]) > f[1] - i
                               for f in fillers)
                    if not need:
                        return
                    while open_g:
                        clock[1] += open_g.pop(0)()
                    for f in fillers:
                        if f[3] is None:
                            continue
                        k = len(f[3]) - max(0, f[1] - i)
                        if k <= 0:
                            continue
                        units, f[3] = f[3], None
                        pe = f[2]
                        for fn in units[:k]:
                            clock[1] += pe
                            fn()
                        if units[k:]:
                            # remainder must stay adjacent in the psum ring:
                            # hand it to the pacer's open group and stop
                            # (deadlines are performance, not correctness)
                            open_g.extend(
                                [lambda fn=fn, pe=pe: (fn(), pe)[1]
                                 for fn in units[k:]])
                            break

                def pace(i, act_ns, pe_ns):
                    clock[0] += act_ns
                    clock[1] += pe_ns
                    # bound the modeled ACT lead: estimation drift must not
                    # turn into a long filler burst that starves the exps
                    clock[0] = min(clock[0], clock[1] + 1200.0)
                    while clock[1] < clock[0]:
                        if open_g:
                            clock[1] += open_g.pop(0)()
                            continue
                        for f in fillers:
                            if f[3] is not None and f[0] <= i:
                                units, f[3] = f[3], None
                                pe = f[2]
                                open_g.extend(
                                    [lambda fn=fn, pe=pe: (fn(), pe)[1]
                                     for fn in units])
                                break
                        else:
                            break

                # software-pipelined: scores run one ski-pair ahead of PV so
                # the PV's wait on exp is hidden behind real PE work
                ctx_tiles = {}
                prev = None
                for i, (j, h, skp) in enumerate(steps):
                    emit_forced(i)
                    if skp == bounds[(j, h)][0]:
                        ctx_tiles[(j, h)] = cpool.tile(
                            [DH + 1, W], f32, tag="ctx", name="ctx_ps")
                    cur = (j, h, skp, emit_scores_pair(j, h, skp))
                    ex0 = max(0, 128 * skp - j * W) if skp >= TPB * j else 0
                    ex1 = (max(0, 128 * (skp + 1) - j * W)
                           if skp + 1 >= TPB * j else 0)
                    act_ns = 2 * (W - ex0) * 0.833 + 110
                    pe_ns = (W - ex0) * cyc + (2 * W - ex0 - ex1) * cyc
                    if prev is not None:
                        pj, ph, pskp, (pattn, pex) = prev
                        pace(i, act_ns, pe_ns)
                        emit_pv_pair(ctx_tiles[(pj, ph)], pj, ph, pskp,
                                     pattn, pex)
                        if pskp + 1 == bounds[(pj, ph)][1]:
                            emit_norm(ctx_tiles[(pj, ph)], pj, ph)
                    prev = cur
                pj, ph, pskp, (pattn, pex) = prev
                emit_pv_pair(ctx_tiles[(pj, ph)], pj, ph, pskp, pattn, pex)
                if tail_pre is not None:
                    tail_pre()
                emit_norm(ctx_tiles[(pj, ph)], pj, ph, halves=2)
                # drain leftover fillers
                while open_g:
                    open_g.pop(0)()
                for f in fillers:
                    if f[3] is not None:
                        units, f[3] = f[3], None
                        for fn in units:
                            fn()

            # ------------- main schedule --------------------------------
            # DMA transfers serialize on one engine in practice, so order
            # them by first use: q weights, x block 0, k weights, ...
            def wdma(n):
                nc.sync.dma_start(w_sb[n][:], w_r[n])

            # block-0 loads ordered by first use; wk ahead of x so the
            # k-group (whose copies gate the first exp) starts immediately
            # after the q-group instead of waiting its weight DMA
            wdma("wqh"), wdma("wql")
            for kq in range(0, KD, 2):
                nc.sync.dma_start(xh_sb[:, kq:kq + 2, 0:W],
                                  xh_r[:, kq:kq + 2, 0:W])
            wdma("wkh"), wdma("wkl")
            wdma("wvh"), wdma("wvl")
            emit_x_dmas(0, "l")
            if NJ > 1:
                emit_x_dmas(1)
            if NJ > 2:
                emit_x_dmas(2)
            nc.sync.dma_start(wo_sb[:], wo.rearrange("(q p) d -> p q d", p=128))
            for j in range(3, NJ):
                emit_x_dmas(j)
            for m in range(MQ):
                emit_qk_group(0, m, "q")
                emit_qk_group(0, m, "k")
            for st in range(TPB):
                emit_v_group(st)
            # last block's output projection: one full-width group per
            # s-tile in the (idle) scores-pool psum, one wide DMA each,
            # alternating copy/DMA engines — minimizes serial HWDGE and
            # DMA hops on the final drain.  The first two groups' plane-0
            # matmuls (heads 0/1, long normalized) run as a tail_pre during
            # the final head's softmax chain.
            tail_tiles = {}

            def tail_pre():
                for st in range((NJ - 1) * TPB, (NJ - 1) * TPB + 2):
                    csl = slice((st % TPB) * 128, (st % TPB) * 128 + 128)
                    ops = spool.tile([128, 2, W], f32, tag="sc",
                                     name="ops_pre")
                    tail_tiles[st] = ops
                    for n in range(2):
                        nc.tensor.matmul(
                            ops[:, n, :],
                            ctx_sb[NJ - 1][:, 0, csl],
                            wo_sb[:, 0, n * 512:(n + 1) * 512],
                            start=True,
                            stop=False,
                        )

            run_attention_steps(tail_pre)
            for i, st in enumerate(range((NJ - 1) * TPB, NJ * TPB)):
                ssl = slice(st * 128, st * 128 + 128)
                csl = slice((st % TPB) * 128, (st % TPB) * 128 + 128)
                ops = tail_tiles.get(st)
                p2s = range(MQ)
                if ops is None:
                    ops = spool.tile([128, 2, W], f32, tag="sc",
                                     name="ops_tail")
                else:
                    p2s = range(1, MQ)
                for n in range(2):
                    for p2 in p2s:
                        nc.tensor.matmul(
                            ops[:, n, :],
                            ctx_sb[NJ - 1][:, p2, csl],
                            wo_sb[:, p2, n * 512:(n + 1) * 512],
                            start=(p2 == 0),
                            stop=(p2 == MQ - 1),
                        )
                o_sb = opool.tile([128, 2, 512], bf16, tag="o2", bufs=4)
                if i % 2 == 0:
                    nc.vector.tensor_copy(o_sb[:], ops[:])
                else:
                    nc.scalar.copy(o_sb[:], ops[:])
                if i % 2 == 1:
                    nc.scalar.dma_start(out[ssl, :], o_sb[:])
                else:
                    nc.sync.dma_start(out[ssl, :], o_sb[:])

    nc.compile()
    return nc


def _get_program():
    key = (S, D, HL, DH)
    if key not in _PROGRAM_CACHE:
        _PROGRAM_CACHE[key] = build_program(*key)
    return _PROGRAM_CACHE[key]


def _split8(a):
    """hi/lo e4m3 split of a float32 array."""
    hi = a.astype(E4)
    lo = (a - hi.astype(np.float32)).astype(E4)
    return hi, lo


def _pack_w(a):
    """[D, M] -> [128, KD*M] partition-contiguous packing (KD = D//128)."""
    D_, M_ = a.shape
    return np.ascontiguousarray(
        a.reshape(D_ // 128, 128, M_).transpose(1, 0, 2).reshape(128, -1))


def prep_core_inputs(xT, Wq, Wk, Wv, Wo, g):
    """Per-core input map.  xT: [D, S] f32 (one batch, transposed);
    W*: full [D, D] f32; g: head-group index (0..NCORES//B-1)."""
    sl = slice(HL * DH * g, HL * DH * (g + 1))
    xh8, xl8 = _split8(xT)
    m = {"xh": xh8, "xl": xl8}
    for name, Wfull in (("wq", Wq), ("wk", Wk), ("wv", Wv)):
        ws = np.ascontiguousarray(Wfull[sl, :].T) * WSC
        hi, lo = _split8(ws)
        m[name + "h"], m[name + "l"] = _pack_w(hi), _pack_w(lo)
    m["wo"] = (np.ascontiguousarray(Wo[:, sl].T) / WSC).astype(BF)
    return m


def make_in_maps(x, Wq, Wk, Wv, Wo):
    x = np.asarray(x, dtype=np.float32)
    Wq = np.asarray(Wq, dtype=np.float32)
    Wk = np.asarray(Wk, dtype=np.float32)
    Wv = np.asarray(Wv, dtype=np.float32)
    Wo = np.asarray(Wo, dtype=np.float32)
    # x split once per batch, shared across the 4 head-group cores
    xs = [_split8(np.ascontiguousarray(x[b].T)) for b in range(B)]
    in_maps = []
    for c in range(NCORES):
        b, g = divmod(c, NCORES // B)
        sl = slice(HL * DH * g, HL * DH * (g + 1))
        m = {"xh": xs[b][0], "xl": xs[b][1]}
        for name, Wfull in (("wq", Wq), ("wk", Wk), ("wv", Wv)):
            ws = np.ascontiguousarray(Wfull[sl, :].T) * WSC
            hi, lo = _split8(ws)
            m[name + "h"], m[name + "l"] = _pack_w(hi), _pack_w(lo)
        m["wo"] = (np.ascontiguousarray(Wo[:, sl].T) / WSC).astype(BF)
        in_maps.append(m)
    return in_maps


def kernel(x, Wq, Wk, Wv, Wo, bo):
    from concourse import bass2jax

    nc = _get_program()
    in_maps = make_in_maps(x, Wq, Wk, Wv, Wo)
    res = bass2jax.run_bass_via_pjrt(nc, in_maps, n_cores=NCORES)
    outs = [np.asarray(res[c]["out"]).astype(np.float32) for c in range(NCORES)]
    gpb = NCORES // B
    o = np.stack([sum(outs[b * gpb + g] for g in range(gpb)) for b in range(B)])
    o = o + np.asarray(bo, dtype=np.float32)[None, None, :]
    return o.astype(np.float32)


# revision 112
# speedup vs baseline: 1.0059x; 1.0059x over previous
"""Trainium2 Bass kernel: multi-head attention forward (B=2, S=2048, D=1024, H=16).

Sharding: 8 cores = data-parallel over batch (2) x tensor-parallel over heads
(4 head-groups of 4 heads).  Host side: inputs are pre-transposed / sliced /
fp8-split per core; the 4 partial outputs per batch are summed and the bias
added on the host (the "unshard").

Precision scheme (measured rel err 1.21e-2 vs the 2e-2 gate):
  - x and Wq/Wk/Wv are hi/lo-split into e4m3 pairs on the host
    (t = hi + lo with hi = fp8(t), lo = fp8(t - hi); residual ~0.1%).
    Weights are pre-scaled by 32 so the lo parts stay in e4m3's normal
    range; the scale is compensated in the exp scale (/1024) and in
    Wo (/32).
  - q/k projections: 2-term fp8 DoubleRow (xh@(Wh+Wl)): exact weights,
    single-fp8 x; q is re-quantized to e4m3 at the scores and k is
    re-split exactly, so an xl term would add nothing.  8 DR
    instructions of N/2 cycles vs 8 of N for f32r (0.5x PE).
  - v projection: 3-term (xh@Wvh + xh@Wvl + xl@Wvh, 0.75x); v errors do
    not average out downstream, so single-fp8 x is NOT acceptable here
    (measured 3e-2).
  - scores: one DoubleRow per sk-tile: stationary slots (kh, kl) so k is
    exact; moving q is a single e4m3 (slot-broadcast AP) -> 0.5x PE cost.
  - exp -> bf16 attention weights (ACT), processed in ski-pairs to halve
    the per-instruction overhead.
  - PV and output projection in bf16 (same PE cost as f32r); bf16 output
    partials, summed in f32 on the host.
  - softmax denominators via a bf16 ones-column in v (row 64 of the PV
    psum); normalization: DVE copy + fast reciprocal + Pool broadcast +
    DVE multiply (DVE divide fails the walrus ISA check; GPSIMD cannot
    read PSUM on hardware).

Scheduling: a global software-pipelined pass over (block, head, ski-pair)
steps: scores+exp run one pair ahead of PV; projection and out-projection
work is split into sub-0.5us filler units paced by a static ACT/PE clock
model so the exp stream (the ~78us ACT floor) never starves; per-block ctx
tiles avoid coarse-dep serialization; DMAs are ordered by first use and
weights host-packed to 2KB-contiguous partition runs (sub-512B runs pay a
2x DMA penalty).
"""

import sys

sys.path.insert(0, "/opt/trn_rl_repo")

import numpy as np
import ml_dtypes

B, S, D = 2, 2048, 1024
H = 16
DH = 64
HL = 4  # heads per core
NCORES = 8

WSC = 32.0  # power-of-2 weight prescale for fp8 range
E4 = ml_dtypes.float8_e4m3
BF = ml_dtypes.bfloat16

_PROGRAM_CACHE = {}


def build_program(S=S, D=D, HL=HL, DH=DH):
    import concourse.tile as tile
    from concourse import bacc, mybir

    f32 = mybir.dt.float32
    f8 = mybir.dt.float8e4
    bf16 = mybir.dt.bfloat16
    A = mybir.ActivationFunctionType
    Alu = mybir.AluOpType
    DR = mybir.MatmulPerfMode.DoubleRow

    K2T = True            # 2-term k-projection (k re-split exactly at scores)
    DIVN = False          # DVE divide fails the walrus ISA check on TRN2
    KD = D // 128         # contraction k-tiles for the projections (8)
    NP = KD // 2          # DoubleRow k-tile pairs (4)
    M = HL * DH           # per-core projected width (256)
    MQ = M // 128         # q/k partition planes (2)
    ST = S // 128         # 128-row s tiles (16)
    W = min(512, S)       # query-block width
    NJ = S // W           # query blocks
    TPB = W // 128        # sk tiles per query block (4)
    scale = (1.0 / float(np.sqrt(DH))) / (WSC * WSC)

    nc = bacc.Bacc("TRN2", target_bir_lowering=False, debug=False)
    xh = nc.dram_tensor("xh", (D, S), f8, kind="ExternalInput").ap()
    xl = nc.dram_tensor("xl", (D, S), f8, kind="ExternalInput").ap()
    # weights pre-packed on the host to [128, KD*M] so each partition's
    # DMA run is 2KB contiguous (256B runs pay a 2x DMA latency penalty)
    wnames = ("wqh", "wql", "wkh", "wkl", "wvh", "wvl")
    wt = {n: nc.dram_tensor(n, (128, KD * M), f8, kind="ExternalInput").ap()
          for n in wnames}
    wo = nc.dram_tensor("wo", (M, D), bf16, kind="ExternalInput").ap()
    # bf16 output: DMA transfers serialize on one engine, so halving the
    # 8MB store matters; the host sums the 4 partials in f32
    out = nc.dram_tensor("out", (S, D), bf16, kind="ExternalOutput").ap()

    xh_r = xh.rearrange("(k p) s -> p k s", p=128)
    xl_r = xl.rearrange("(k p) s -> p k s", p=128)
    w_r = {n: wt[n].rearrange("p (k m) -> p k m", k=KD) for n in wnames}

    with tile.TileContext(nc) as tc:
        with (
            tc.tile_pool(name="persist", bufs=1) as mp,
            tc.tile_pool(name="attn", bufs=4) as apool,
            tc.tile_pool(name="norm", bufs=2) as rpool,
            tc.tile_pool(name="ostage", bufs=8) as opool,
            tc.tile_pool(name="gps", bufs=2, space="PSUM") as gpool,
            tc.tile_pool(name="scps", bufs=2, space="PSUM") as spool,
            tc.tile_pool(name="ctxps", bufs=2, space="PSUM") as cpool,
        ):
            xh_sb = mp.tile([128, KD, S], f8, tag="xh")
            xl_sb = mp.tile([128, KD, S], f8, tag="xl")
            w_sb = {n: mp.tile([128, KD, M], f8, tag=n, name=n) for n in w_r}
            wo_sb = mp.tile([128, MQ, D], bf16, tag="wo")
            q8_sb = mp.tile([128, MQ, S], f8, tag="q8")
            k8_sb = mp.tile([128, MQ, 2, S], f8, tag="k8")
            v_sb = mp.tile([128, ST, HL * (DH + 1)], bf16, tag="v")
            # per-block ctx tiles: the dep tracker is coarse per tile, so a
            # single [128, MQ, S] tile would serialize block j's out-proj
            # behind block j+1's normalization writes
            ctx_sb = [
                mp.tile([128, MQ, W], bf16, tag=f"ctx{j}", name=f"ctx{j}")
                for j in range(NJ)
            ]

            # ones columns for the PV denominator trick
            nc.vector.memset(v_sb[:, :, DH::DH + 1], 1.0)

            # pstate warmup: dummy matmuls while the first DMAs are in
            # flight, so real work starts at full PE clock
            wu = mp.tile([128, 2, 128], f8, tag="wu")
            nc.vector.memset(wu[:], 0.0)
            wu_ps = gpool.tile([128, 512], f32, tag="ps", name="wu_ps")
            nc.tensor.matmul(
                wu_ps[:, 0:128],
                wu[:, :, :],
                wu[:, :, :],
                start=True, stop=True, perf_mode=DR,
            )

            # ------------- per-block emitters ---------------------------
            def emit_x_dmas(j, part="hl"):
                jsl = slice(j * W, (j + 1) * W)
                if "h" in part:
                    nc.sync.dma_start(xh_sb[:, :, jsl], xh_r[:, :, jsl])
                if "l" in part:
                    nc.sync.dma_start(xl_sb[:, :, jsl], xl_r[:, :, jsl])

            def qk_units(j, m, dst, q8_on_act=False):
                """The q/k projection group as two half units sharing one
                psum accumulation (filler granularity below the per-pair PE
                slack, so emitting one never starves the exp stream)."""
                jsl = slice(j * W, (j + 1) * W)
                msl = slice(m * 128, (m + 1) * 128)
                wh, wl = ("wqh", "wql") if dst == "q" else ("wkh", "wkl")
                state = {}

                def unit(u):
                    if u == 0:
                        state["ps"] = gpool.tile([128, W], f32, tag="ps",
                                                 name="ps_qk")
                    ps = state["ps"]
                    # 2-term projection (exact W, single-fp8 x): q is
                    # re-quantized to e4m3 at the scores and k is re-split
                    # exactly, so the xl term would be wasted effort
                    i = 4 * u
                    for p in (2 * u, 2 * u + 1):
                        psl = slice(2 * p, 2 * p + 2)
                        for sw in (wh, wl):
                            nc.tensor.matmul(
                                ps[:],
                                w_sb[sw][:, psl, msl],
                                xh_sb[:, psl, jsl],
                                start=(i == 0),
                                stop=(i == 2 * NP - 1),
                                perf_mode=DR,
                            )
                            i += 1
                    if u == 1:
                        if dst == "q":
                            if q8_on_act:
                                # ACT is idle before the exp stream starts:
                                # unserializes the first q8/kh/kl copies
                                nc.scalar.copy(q8_sb[:, m, jsl], ps[:])
                            else:
                                nc.vector.tensor_copy(q8_sb[:, m, jsl], ps[:])
                        else:
                            nc.vector.tensor_copy(k8_sb[:, m, 0, jsl], ps[:])
                            nc.vector.tensor_sub(
                                k8_sb[:, m, 1, jsl], ps[:],
                                k8_sb[:, m, 0, jsl],
                            )

                return [lambda u=u: unit(u) for u in range(2)]

            def v_units(st):
                ssl = slice(st * 128, st * 128 + 128)
                terms = [(xh_sb, "wvh"), (xh_sb, "wvl"), (xl_sb, "wvh")]
                state = {}

                def unit(u):
                    if u == 0:
                        state["ps"] = gpool.tile([128, M], f32, tag="ps",
                                                 name="psv")
                    psv = state["ps"]
                    todo = [(t, p) for t in range(3) for p in range(NP)]
                    for t, p in todo[6 * u:6 * (u + 1)]:
                        sx, sw = terms[t]
                        psl = slice(2 * p, 2 * p + 2)
                        nc.tensor.matmul(
                            psv[:],
                            sx[:, psl, ssl],
                            w_sb[sw][:, psl, :],
                            start=(t == 0 and p == 0),
                            stop=(t == 2 and p == NP - 1),
                            perf_mode=DR,
                        )
                    if u == 1:
                        vdst = v_sb[:, st].rearrange(
                            "p (h c) -> p h c", h=HL)[:, :, 0:DH]
                        nc.vector.tensor_copy(
                            vdst, psv[:].rearrange("p (h c) -> p h c", h=HL))

                return [lambda u=u: unit(u) for u in range(2)]

            def emit_qk_group(j, m, dst, q8_on_act=False):
                for fn in qk_units(j, m, dst, q8_on_act):
                    fn()

            def emit_v_group(st):
                for fn in v_units(st):
                    fn()

            def emit_outproj_group(st, n, copy_dve=True, dma_act=False,
                                   nw=512, pool=None):
                ssl = slice(st * 128, st * 128 + 128)
                csl = slice((st % TPB) * 128, (st % TPB) * 128 + 128)
                nsl = slice(n * nw, (n + 1) * nw)
                if pool is None:
                    ops = gpool.tile([128, 512], f32, tag="ps", name="ops")
                else:
                    # tail: borrow the (idle) scores-pool slots so the drain
                    # has a 4-deep psum ring instead of 2
                    ops = pool.tile([128, 2, W], f32, tag="sc",
                                    name="ops_sc")[:, 0, :]
                for p2 in range(MQ):
                    nc.tensor.matmul(
                        ops[:, 0:nw],
                        ctx_sb[st // TPB][:, p2, csl],
                        wo_sb[:, p2, nsl],
                        start=(p2 == 0),
                        stop=(p2 == MQ - 1),
                    )
                # GPSIMD cannot read PSUM on hardware: stage on DVE (or ACT
                # for the drain, where the exps are done)
                o_sb = opool.tile([128, 512], bf16, tag="o")
                if copy_dve:
                    nc.vector.tensor_copy(o_sb[:, 0:nw], ops[:, 0:nw])
                else:
                    nc.scalar.copy(o_sb[:, 0:nw], ops[:, 0:nw])
                if dma_act:
                    nc.scalar.dma_start(out[ssl, nsl], o_sb[:, 0:nw])
                else:
                    nc.sync.dma_start(out[ssl, nsl], o_sb[:, 0:nw])

            # fillers: PE work (next blocks' projections, previous blocks'
            # output projection) with deadlines, spread evenly over the
            # global attention pair-step sequence to fill PE stalls on ACT
            def run_attention_steps(tail_pre=None):
                def emit_scores_pair(j, h, skp):
                    """scores + exp + causal mask for one ski-pair; returns
                    what the (deferred) PV pair needs."""
                    hm, po = h // 2, 64 * (h % 2)
                    qrow = slice(po, po + DH)
                    sc = spool.tile([128, 2, W], f32, tag="sc")
                    ex = [
                        max(0, 128 * (skp + t) - j * W)
                        if skp + t >= TPB * j else 0
                        for t in range(2)
                    ]
                    for t in range(2):
                        ski = skp + t
                        # both slots cover the pair's union range so the
                        # paired exp reads fully-written psum; the extra
                        # columns in slot 1 are causally dead (PV skips)
                        q_mov = q8_sb[qrow, hm:hm + 1,
                                      j * W + ex[0]:(j + 1) * W]
                        nc.tensor.matmul(
                            sc[:, t, ex[0]:W],
                            k8_sb[qrow, hm, :, ski * 128:ski * 128 + 128],
                            q_mov.broadcast_to((DH, 2, W - ex[0])),
                            start=True,
                            stop=True,
                            perf_mode=DR,
                        )
                    attn = apool.tile([128, 2, W], bf16, tag="attn")
                    nc.scalar.activation(
                        attn[:, :, ex[0]:W], sc[:, :, ex[0]:W], A.Exp,
                        scale=scale,
                    )
                    for t in range(2):
                        ski = skp + t
                        if ski >= TPB * j:  # diagonal-crossing tile
                            cross_end = 128 * ski + 128 - j * W
                            nc.gpsimd.affine_select(
                                out=attn[:, t, ex[t]:cross_end],
                                in_=attn[:, t, ex[t]:cross_end],
                                compare_op=Alu.is_ge,
                                fill=0.0,
                                base=j * W + ex[t] - 128 * ski,
                                pattern=[[1, cross_end - ex[t]]],
                                channel_multiplier=-1,
                            )
                    return attn, ex

                def emit_pv_pair(ctx_ps, j, h, skp, attn, ex):
                    hv = slice(h * (DH + 1), (h + 1) * (DH + 1))
                    fs, ls = bounds[(j, h)]
                    for t in range(2):
                        ski = skp + t
                        nc.tensor.matmul(
                            ctx_ps[:, ex[t]:W],
                            v_sb[:, ski, hv],
                            attn[:, t, ex[t]:W],
                            start=(ski == fs),
                            stop=(ski == ls),
                        )

                def emit_norm(ctx_ps, j, h, halves=1):
                    hm, po = h // 2, 64 * (h % 2)
                    hw = W // halves
                    for c in range(halves):
                        cs = slice(c * hw, (c + 1) * hw)
                        dcp = rpool.tile([1, W], f32, tag="d")
                        rcp = rpool.tile([1, W], f32, tag="r")
                        bc = rpool.tile([64, W], f32, tag="bc")
                        nc.vector.tensor_copy(dcp[:, 0:hw], ctx_ps[DH:DH + 1, cs])
                        nc.vector.reciprocal_approx_fast(
                            out=rcp[:, 0:hw], in_=dcp[:, 0:hw])
                        nc.gpsimd.partition_broadcast(
                            bc[:, 0:hw], rcp[:, 0:hw], channels=64)
                        nc.vector.tensor_mul(
                            ctx_sb[j][po:po + DH, hm, cs], ctx_ps[0:DH, cs],
                            bc[:, 0:hw],
                        )

                # global step list over (block, head, ski-pair).  For the
                # final head the diagonal pairs go FIRST (psum accumulation
                # order is free) so no affine_select sits on the critical
                # chain right before the last PV + softmax.
                steps = []
                bounds = {}
                for j in range(NJ):
                    for h in range(HL):
                        prs = list(range(0, TPB * (j + 1), 2))
                        if (j, h) == (NJ - 1, HL - 1) and j > 0:
                            d = TPB * j // 2
                            prs = prs[d:] + prs[:d]
                        bounds[(j, h)] = (prs[0], prs[-1] + 1)
                        steps += [(j, h, skp) for skp in prs]
                sidx = {s: i for i, s in enumerate(steps)}
                n_steps = len(steps)

                # fillers: (earliest, deadline, unit_pe_ns, [unit closures]);
                # block 0's second-plane q/k and v groups are fillers too so
                # the first scores (and the exp stream) start sooner.
                # Pacing tracks a static model of the ACT and PE clocks and
                # emits one sub-slack-sized unit whenever PE would otherwise
                # run ahead of the exp stream.  A group's units share a psum
                # tile, so once started a group is finished before any other
                # filler (nothing else touches that pool between pace calls).
                cyc = 1.0 / 2.4
                qk_ns = 4 * 256 * cyc
                v_ns = 6 * 128 * cyc
                op_ns = 2 * 512 * cyc
                fillers = []
                for j in range(1, NJ):
                    e0 = sidx[(j - 1, 0, 0)]
                    for m in range(MQ):
                        dl = sidx[(j, 2 * m, 0)]
                        fillers.append([e0, dl, qk_ns, qk_units(j, m, "q")])
                        fillers.append([e0, dl, qk_ns, qk_units(j, m, "k")])
                    for st in range(j * TPB, (j + 1) * TPB):
                        dl = sidx[(j, 0, st - st % 2)] + 1
                        fillers.append([e0, dl, v_ns, v_units(st)])
                for j in range(NJ - 1):
                    # not at (j+1, 0, 0): block j's last norm-mul is emitted
                    # there, and an out-proj filler would head-of-line block
                    # the PE queue waiting for it, starving the exp stream
                    e0 = sidx[(j + 1, 1, 0)]
                    for st in range(j * TPB, (j + 1) * TPB):
                        for n in range(D // 512):
                            fillers.append([e0, n_steps, op_ns,
                                            [lambda st=st, n=n:
                                             emit_outproj_group(st, n)]])
                fillers.sort(key=lambda f: f[1])
                # PE starts the step loop still working off the upfront
                # block-0 projections; pre-load its clock so early pacing
                # does not over-fill and starve the exp stream
                clock = [0.0, 4 * 2 * qk_ns + 4 * 2 * v_ns]
                open_g = []  # remaining units of the group being emitted

                def emit_forced(i):
                    # spread deadline work: emit only as many units per step
                    # as needed to make each deadline, instead of dumping
                    # whole groups in one burst that starves the exp stream
                    need = any(f[3] is not None and len(f[3]) > f[1] - i
                               for f in fillers)
                    if not need:
                        return
                    while open_g:
                        clock[1] += open_g.pop(0)()
                    for f in fillers:
                        if f[3] is None:
                            continue
                        k = len(f[3]) - max(0, f[1] - i)
                        if k <= 0:
                            continue
                        units, f[3] = f[3], None
                        pe = f[2]
                        for fn in units[:k]:
                            clock[1] += pe
                            fn()
                        if units[k:]:
                            # remainder must stay adjacent in the psum ring:
                            # hand it to the pacer's open group and stop
                            # (deadlines are performance, not correctness)
                            open_g.extend(
                                [lambda fn=fn, pe=pe: (fn(), pe)[1]
                                 for fn in units[k:]])
                            break

                def pace(i, act_ns, pe_ns):
                    clock[0] += act_ns
                    clock[1] += pe_ns
                    # bound the modeled ACT lead: estimation drift must not
                    # turn into a long filler burst that starves the exps
                    clock[0] = min(clock[0], clock[1] + 1200.0)
                    while clock[1] < clock[0]:
                        if open_g:
                            clock[1] += open_g.pop(0)()
                            continue
                        for f in fillers:
                            if f[3] is not None and f[0] <= i:
                                units, f[3] = f[3], None
                                pe = f[2]
                                open_g.extend(
                                    [lambda fn=fn, pe=pe: (fn(), pe)[1]
                                     for fn in units])
                                break
                        else:
                            break

                # software-pipelined: scores run one ski-pair ahead of PV so
                # the PV's wait on exp is hidden behind real PE work
                ctx_tiles = {}
                prev = None
                for i, (j, h, skp) in enumerate(steps):
                    emit_forced(i)
                    if skp == bounds[(j, h)][0]:
                        ctx_tiles[(j, h)] = cpool.tile(
                            [DH + 1, W], f32, tag="ctx", name="ctx_ps")
                    cur = (j, h, skp, emit_scores_pair(j, h, skp))
                    ex0 = max(0, 128 * skp - j * W) if skp >= TPB * j else 0
                    ex1 = (max(0, 128 * (skp + 1) - j * W)
                           if skp + 1 >= TPB * j else 0)
                    act_ns = 2 * (W - ex0) * 0.833 + 110
                    pe_ns = (W - ex0) * cyc + (2 * W - ex0 - ex1) * cyc
                    if prev is not None:
                        pj, ph, pskp, (pattn, pex) = prev
                        pace(i, act_ns, pe_ns)
                        emit_pv_pair(ctx_tiles[(pj, ph)], pj, ph, pskp,
                                     pattn, pex)
                        if pskp + 1 == bounds[(pj, ph)][1]:
                            emit_norm(ctx_tiles[(pj, ph)], pj, ph)
                    prev = cur
                pj, ph, pskp, (pattn, pex) = prev
                emit_pv_pair(ctx_tiles[(pj, ph)], pj, ph, pskp, pattn, pex)
                if tail_pre is not None:
                    tail_pre()
                emit_norm(ctx_tiles[(pj, ph)], pj, ph, halves=2)
                # drain leftover fillers
                while open_g:
                    open_g.pop(0)()
                for f in fillers:
                    if f[3] is not None:
                        units, f[3] = f[3], None
                        for fn in units:
                            fn()

            # ------------- main schedule --------------------------------
            # DMA transfers serialize on one engine in practice, so order
            # them by first use: q weights, x block 0, k weights, ...
            def wdma(n):
                nc.sync.dma_start(w_sb[n][:], w_r[n])

            # block-0 loads ordered by first use; wk ahead of x so the
            # k-group (whose copies gate the first exp) starts immediately
            # after the q-group instead of waiting its weight DMA
            wdma("wqh"), wdma("wql")
            nc.sync.dma_start(xh_sb[:, 0:KD // 2, 0:W], xh_r[:, 0:KD // 2, 0:W])
            nc.sync.dma_start(xh_sb[:, KD // 2:KD, 0:W],
                              xh_r[:, KD // 2:KD, 0:W])
            wdma("wkh"), wdma("wkl")
            wdma("wvh"), wdma("wvl")
            emit_x_dmas(0, "l")
            if NJ > 1:
                emit_x_dmas(1)
            if NJ > 2:
                emit_x_dmas(2)
            nc.sync.dma_start(wo_sb[:], wo.rearrange("(q p) d -> p q d", p=128))
            for j in range(3, NJ):
                emit_x_dmas(j)
            for m in range(MQ):
                emit_qk_group(0, m, "q")
                emit_qk_group(0, m, "k")
            for st in range(TPB):
                emit_v_group(st)
            # last block's output projection: one full-width group per
            # s-tile in the (idle) scores-pool psum, one wide DMA each,
            # alternating copy/DMA engines — minimizes serial HWDGE and
            # DMA hops on the final drain.  The first two groups' plane-0
            # matmuls (heads 0/1, long normalized) run as a tail_pre during
            # the final head's softmax chain.
            tail_tiles = {}

            def tail_pre():
                for st in range((NJ - 1) * TPB, (NJ - 1) * TPB + 2):
                    csl = slice((st % TPB) * 128, (st % TPB) * 128 + 128)
                    ops = spool.tile([128, 2, W], f32, tag="sc",
                                     name="ops_pre")
                    tail_tiles[st] = ops
                    for n in range(2):
                        nc.tensor.matmul(
                            ops[:, n, :],
                            ctx_sb[NJ - 1][:, 0, csl],
                            wo_sb[:, 0, n * 512:(n + 1) * 512],
                            start=True,
                            stop=False,
                        )

            run_attention_steps(tail_pre)
            for i, st in enumerate(range((NJ - 1) * TPB, NJ * TPB)):
                ssl = slice(st * 128, st * 128 + 128)
                csl = slice((st % TPB) * 128, (st % TPB) * 128 + 128)
                ops = tail_tiles.get(st)
                p2s = range(MQ)
                if ops is None:
                    ops = spool.tile([128, 2, W], f32, tag="sc",
                                     name="ops_tail")
                else:
                    p2s = range(1, MQ)
                for n in range(2):
                    for p2 in p2s:
                        nc.tensor.matmul(
                            ops[:, n, :],
                            ctx_sb[NJ - 1][:, p2, csl],
                            wo_sb[:, p2, n * 512:(n + 1) * 512],
                            start=(p2 == 0),
                            stop=(p2 == MQ - 1),
                        )
                o_sb = opool.tile([128, 2, 512], bf16, tag="o2", bufs=4)
                if i % 2 == 0:
                    nc.vector.tensor_copy(o_sb[:], ops[:])
                else:
                    nc.scalar.copy(o_sb[:], ops[:])
                if i % 2 == 1:
                    nc.scalar.dma_start(out[ssl, :], o_sb[:])
                else:
                    nc.sync.dma_start(out[ssl, :], o_sb[:])

    nc.compile()
    return nc


def _get_program():
    key = (S, D, HL, DH)
    if key not in _PROGRAM_CACHE:
        _PROGRAM_CACHE[key] = build_program(*key)
    return _PROGRAM_CACHE[key]


def _split8(a):
    """hi/lo e4m3 split of a float32 array."""
    hi = a.astype(E4)
    lo = (a - hi.astype(np.float32)).astype(E4)
    return hi, lo


def _pack_w(a):
    """[D, M] -> [128, KD*M] partition-contiguous packing (KD = D//128)."""
    D_, M_ = a.shape
    return np.ascontiguousarray(
        a.reshape(D_ // 128, 128, M_).transpose(1, 0, 2).reshape(128, -1))


def prep_core_inputs(xT, Wq, Wk, Wv, Wo, g):
    """Per-core input map.  xT: [D, S] f32 (one batch, transposed);
    W*: full [D, D] f32; g: head-group index (0..NCORES//B-1)."""
    sl = slice(HL * DH * g, HL * DH * (g + 1))
    xh8, xl8 = _split8(xT)
    m = {"xh": xh8, "xl": xl8}
    for name, Wfull in (("wq", Wq), ("wk", Wk), ("wv", Wv)):
        ws = np.ascontiguousarray(Wfull[sl, :].T) * WSC
        hi, lo = _split8(ws)
        m[name + "h"], m[name + "l"] = _pack_w(hi), _pack_w(lo)
    m["wo"] = (np.ascontiguousarray(Wo[:, sl].T) / WSC).astype(BF)
    return m


def make_in_maps(x, Wq, Wk, Wv, Wo):
    x = np.asarray(x, dtype=np.float32)
    Wq = np.asarray(Wq, dtype=np.float32)
    Wk = np.asarray(Wk, dtype=np.float32)
    Wv = np.asarray(Wv, dtype=np.float32)
    Wo = np.asarray(Wo, dtype=np.float32)
    # x split once per batch, shared across the 4 head-group cores
    xs = [_split8(np.ascontiguousarray(x[b].T)) for b in range(B)]
    in_maps = []
    for c in range(NCORES):
        b, g = divmod(c, NCORES // B)
        sl = slice(HL * DH * g, HL * DH * (g + 1))
        m = {"xh": xs[b][0], "xl": xs[b][1]}
        for name, Wfull in (("wq", Wq), ("wk", Wk), ("wv", Wv)):
            ws = np.ascontiguousarray(Wfull[sl, :].T) * WSC
            hi, lo = _split8(ws)
            m[name + "h"], m[name + "l"] = _pack_w(hi), _pack_w(lo)
        m["wo"] = (np.ascontiguousarray(Wo[:, sl].T) / WSC).astype(BF)
        in_maps.append(m)
    return in_maps


def kernel(x, Wq, Wk, Wv, Wo, bo):
    from concourse import bass2jax

    nc = _get_program()
    in_maps = make_in_maps(x, Wq, Wk, Wv, Wo)
    res = bass2jax.run_bass_via_pjrt(nc, in_maps, n_cores=NCORES)
    outs = [np.asarray(res[c]["out"]).astype(np.float32) for c in range(NCORES)]
    gpb = NCORES // B
    o = np.stack([sum(outs[b * gpb + g] for g in range(gpb)) for b in range(B)])
    o = o + np.asarray(bo, dtype=np.float32)[None, None, :]
    return o.astype(np.float32)
